# revision 1
# baseline (speedup 1.0000x reference)
"""Trainium2 Bass kernel for a binarized MLP (BNN) trained-mode forward pass.

Computation (reference):
    h = sign(BN(x @ sign(W1).T)); h = sign(BN(h @ sign(W2).T));
    h = sign(BN(h @ sign(W3).T)); out = h @ sign(W4).T
with BatchNorm1d in training mode (batch stats over the full 16384 batch),
gamma = 1, beta = 0.

Numerical design (the kernel is bit-exact vs the fp64 pipeline):
  * With gamma > 0 and beta == 0, sign(BN(y)) == sign(y - mean(y)) -- the
    variance never matters.
  * Layers 2..4 operate on +-1 activations and +-1/0 weights: fp8e4m3
    represents +-1 exactly, products are exact, and accumulation stays in
    integer range far below 2^24, so those layers are EXACT. fp8 DoubleRow
    runs the PE at ~1.4x bf16 speed.
  * Layer 1's fp32 x is split into three bf16 terms (x == x0+x1+x2 exactly),
    so the layer-1 matmul has full fp32 fidelity at bf16 speed (3 passes).
  * Batch means for layers 2/3 derive from per-feature row sums of the +-1
    activations: mean(h @ S.T) == (S @ rowsum(h)) / B, exact in integers.
    Only two 4 KB all-reduces are needed; both are hidden behind matmul work
    by staging early PSUM chunks to SBUF (plain copy, no mean needed) and
    signing them after the all-reduce lands.
  * Layer-1's mean is computed on the host in fp64 (exact distributivity:
    mean(x @ S1.T) == S1 @ mean(x)), so layer 1 needs no collective.

Sharding: data-parallel over the batch (16384 / 8 = 2048 rows per core),
weights replicated. Activations live on-chip in [feature, batch] layout so
the BN+binarize is one fused ScalarE op: Sign(y + bias), bias = -mu per
partition, with the free-axis row sums produced by the same instruction
(accum_out).
"""

import numpy as np
import ml_dtypes

BF16 = ml_dtypes.bfloat16
FP8 = ml_dtypes.float8_e4m3

N_CORES = 8
B_FULL = 16384
B_SHARD = B_FULL // N_CORES  # 2048
D_IN = 784
K1 = 896  # 784 padded to a multiple of 128
D = 1024
D_OUT = 10
NCHUNK = 512
N_T = B_SHARD // NCHUNK  # 4
M_T = D // 128  # 8
K1_T = K1 // 128  # 7
A_T = D // 256  # 4 DoubleRow blocks of 256 contraction rows

_PROGRAM = None
LAST_RESULTS = None  # BassKernelResults of the most recent device run


def _build_program(debug_outputs=False):
    from concourse import bacc
    import concourse.tile as tile
    import concourse.mybir as mybir

    f32 = mybir.dt.float32
    bf16 = mybir.dt.bfloat16
    fp8 = mybir.dt.float8e4
    AF = mybir.ActivationFunctionType
    AX = mybir.AxisListType.X
    DR = mybir.MatmulPerfMode.DoubleRow

    nc = bacc.Bacc(
        "TRN2", target_bir_lowering=False, debug=False, num_devices=N_CORES
    )

    xs_d = nc.dram_tensor("xs", [3, K1, B_SHARD], bf16, kind="ExternalInput").ap()
    s1t_d = nc.dram_tensor("s1t", [K1, D], bf16, kind="ExternalInput").ap()
    s2b_d = nc.dram_tensor("s2b", [D, D], bf16, kind="ExternalInput").ap()
    s3b_d = nc.dram_tensor("s3b", [D, D], bf16, kind="ExternalInput").ap()
    s2dr_d = nc.dram_tensor("s2dr", [A_T, 128, 2, D], fp8, kind="ExternalInput").ap()
    s3dr_d = nc.dram_tensor("s3dr", [A_T, 128, 2, D], fp8, kind="ExternalInput").ap()
    # last dim padded 10 -> 16: DoubleRow LDWEIGHTS needs the pair-axis
    # step to be 16-byte aligned
    s4dr_d = nc.dram_tensor(
        "s4dr", [A_T, 128, 2, 16], fp8, kind="ExternalInput"
    ).ap()
    nmu1_d = nc.dram_tensor("negmu1", [128, M_T], f32, kind="ExternalInput").ap()
    out_d = nc.dram_tensor("out", [D_OUT, B_SHARD], f32, kind="ExternalOutput").ap()
    dbg = {}
    if debug_outputs:
        for nm, shape, dt_ in [
            ("dbg_h1", [128, M_T, B_SHARD], fp8),
            ("dbg_h2", [128, M_T, B_SHARD], fp8),
            ("dbg_nmu2", [128, M_T], f32),
            ("dbg_nmu3", [128, M_T], f32),
        ]:
            dbg[nm] = nc.dram_tensor(nm, shape, dt_, kind="ExternalOutput").ap()

    with tile.TileContext(nc) as tc:
        with (
            tc.tile_pool(name="w", bufs=1) as wp,
            tc.tile_pool(name="xb", bufs=2) as xp,
            tc.tile_pool(name="h", bufs=1) as hp,
            tc.tile_pool(name="h3", bufs=4) as h3p_pool,
            tc.tile_pool(name="stg", bufs=30) as stp,
            tc.tile_pool(name="small", bufs=1) as sp,
            tc.tile_pool(name="ob", bufs=2) as op_,
            tc.tile_pool(name="yps", bufs=6, space="PSUM") as yp,
            tc.tile_pool(name="mups", bufs=1, space="PSUM") as mp,
            tc.tile_pool(name="l4ps", bufs=1, space="PSUM") as lp,
            tc.tile_pool(name="dram", bufs=1, space="DRAM") as dp,
        ):
            # ---- leading barrier: absorb cross-core launch skew before the
            # compute starts so the later all-reduces see only drift --------
            bar = dp.tile([128, 1], f32, tag="bar", name="bar")
            bar_o = dp.tile([128, 1], f32, tag="bar_o", name="bar_o")
            bar_s = sp.tile([128, 1], f32, tag="bar_s", name="bar_s")
            nc.gpsimd.memset(bar_s[:], 0.0)
            nc.gpsimd.dma_start(out=bar[:], in_=bar_s[:])
            nc.gpsimd.collective_compute(
                "AllReduce",
                mybir.AluOpType.add,
                replica_groups=[list(range(N_CORES))],
                ins=[bar[:].opt()],
                outs=[bar_o[:].opt()],
            )
            # ---- layer-1 weights + x(n=0), interleaved so the PE can start
            # as soon as the first (k, split) pair lands -------------------
            nmu1 = sp.tile([128, M_T], f32, tag="nmu1", name="nmu1")
            nc.sync.dma_start(out=nmu1[:], in_=nmu1_d)
            s1w = [
                wp.tile([128, D], bf16, tag=f"s1_{k}", name=f"s1w{k}")
                for k in range(K1_T)
            ]
            xc = [None] * N_T

            def load_x(n, split_dmas):
                """One [128, 21, 512] tile holding all (split, k) planes of
                batch chunk n -- a single DMA (or 3 on the first chunk so the
                PE can start sooner), so the 21-matmul groups reading it cost
                one semaphore wait instead of 21."""
                t = xp.tile(
                    [128, 3 * K1_T, NCHUNK], bf16, tag="x", name=f"x_{n}"
                )
                nsl = slice(n * NCHUNK, (n + 1) * NCHUNK)
                if split_dmas:
                    for s in range(3):
                        nc.sync.dma_start(
                            out=t[:, s * K1_T : (s + 1) * K1_T, :],
                            in_=xs_d[s].rearrange("(kt p) b -> p kt b", p=128)[
                                :, :, nsl
                            ],
                        )
                else:
                    nc.sync.dma_start(
                        out=t[:],
                        in_=xs_d.rearrange("s (kt p) b -> p (s kt) b", p=128)[
                            :, :, nsl
                        ],
                    )
                xc[n] = t

            for k in range(K1_T):
                nc.sync.dma_start(out=s1w[k][:], in_=s1t_d[k * 128 : (k + 1) * 128, :])
            load_x(0, split_dmas=True)

            # h pairs in fp8 (DoubleRow rhs layout: plane j of pair a is
            # feature tile m = 2a + j)
            h1p = [
                hp.tile([128, 2, B_SHARD], fp8, tag=f"h1_{a}", name=f"h1_{a}")
                for a in range(A_T)
            ]
            h2p = [
                hp.tile([128, 2, B_SHARD], fp8, tag=f"h2_{a}", name=f"h2_{a}")
                for a in range(A_T)
            ]
            hacc1 = sp.tile([128, M_T * N_T], f32, tag="hacc1", name="hacc1")
            hacc2 = sp.tile([128, M_T * N_T], f32, tag="hacc2", name="hacc2")

            # ---- layer 1 ----------------------------------------------
            for n in range(N_T):
                nsl = slice(n * NCHUNK, (n + 1) * NCHUNK)
                if n > 0:
                    load_x(n, split_dmas=False)
                for m0 in range(0, M_T, 2):
                    # two feature tiles interleaved: consecutive matmuls hit
                    # different PSUM banks, so array drains overlap
                    pss = [
                        yp.tile([128, NCHUNK], f32, tag="y", name=f"ps1_{n}_{m0 + j}")
                        for j in range(2)
                    ]
                    idx = 0
                    for s in range(3):
                        for k in range(K1_T):
                            for j in range(2):
                                msl = slice((m0 + j) * 128, (m0 + j + 1) * 128)
                                nc.tensor.matmul(
                                    pss[j][:],
                                    s1w[k][:, msl],
                                    xc[n][:, s * K1_T + k, :],
                                    start=(idx == 0),
                                    stop=(idx == 3 * K1_T - 1),
                                )
                            idx += 1
                    for j in range(2):
                        m = m0 + j
                        c = m * N_T + n
                        nc.scalar.activation(
                            h1p[m // 2][:, m % 2, nsl],
                            pss[j][:],
                            AF.Sign,
                            bias=nmu1[:, m : m + 1],
                            accum_out=hacc1[:, c : c + 1],
                        )

            # ---- later-layer weights (emitted after L1 so their DMAs don't
            # delay the x stream) ---------------------------------------
            def load_ktiles(dram_ap, tagp):
                ts = []
                for k in range(M_T):
                    t = wp.tile([128, D], bf16, tag=f"{tagp}_{k}", name=f"{tagp}{k}")
                    nc.sync.dma_start(
                        out=t[:], in_=dram_ap[k * 128 : (k + 1) * 128, :]
                    )
                    ts.append(t)
                return ts

            def load_dr(dram_ap, tagp, dout):
                ts = []
                for a in range(A_T):
                    t = wp.tile([128, 2, dout], fp8, tag=f"{tagp}_{a}", name=f"{tagp}{a}")
                    nc.sync.dma_start(out=t[:], in_=dram_ap[a])
                    ts.append(t)
                return ts

            s2b = load_ktiles(s2b_d, "s2b")
            s2dr = load_dr(s2dr_d, "s2dr", D)
            s3b = load_ktiles(s3b_d, "s3b")
            s3dr = load_dr(s3dr_d, "s3dr", D)
            s4dr = load_dr(s4dr_d, "s4dr", 16)

            if debug_outputs:
                for a in range(A_T):
                    for j in range(2):
                        nc.sync.dma_start(
                            out=dbg["dbg_h1"][:, 2 * a + j, :], in_=h1p[a][:, j, :]
                        )

            # ---- stats pipeline ---------------------------------------
            # Per-feature batch means for the next layer, computed from the
            # LOCAL +-1 rowsums: mean(h @ S.T) = (S @ rowsum(h)) / B, and by
            # linearity the all-reduce can run on the matvec OUTPUT -- so the
            # PE-side mean matmul needs nothing from the collective, and only
            # the Sign bias waits for it. Everything stays exact integers.
            def stats(hacc, sbw, lname):
                hs = sp.tile([128, M_T], f32, tag=f"hs{lname}", name=f"hs{lname}")
                nc.vector.reduce_sum(
                    hs[:], hacc[:].rearrange("p (m n) -> p m n", n=N_T), axis=AX
                )
                # exact split of local integer rowsums (<= 2048) into two
                # bf16 halves for the mean matmul
                hh = sp.tile([128, M_T, 2], bf16, tag=f"hh{lname}", name=f"hh{lname}")
                hhif = sp.tile(
                    [128, M_T], f32, tag=f"hhif{lname}", name=f"hhif{lname}"
                )
                nc.vector.tensor_copy(hh[:, :, 0], hs[:])
                nc.vector.tensor_copy(hhif[:], hh[:, :, 0])
                nc.vector.tensor_sub(hh[:, :, 1], hs[:], hhif[:])
                pmu = mp.tile([128, M_T, 2], f32, tag="mu", name=f"pmu{lname}")
                for m in range(M_T):
                    msl = slice(m * 128, (m + 1) * 128)
                    for k in range(M_T):
                        nc.tensor.matmul(
                            pmu[:, m, :],
                            sbw[k][:, msl],
                            hh[:, k, :],
                            start=(k == 0),
                            stop=(k == M_T - 1),
                        )
                pmus = sp.tile(
                    [128, M_T, 2], f32, tag=f"pmus{lname}", name=f"pmus{lname}"
                )
                nc.vector.tensor_copy(pmus[:], pmu[:])
                pml = sp.tile([128, M_T], f32, tag=f"pml{lname}", name=f"pml{lname}")
                nc.vector.tensor_add(pml[:], pmus[:, :, 0], pmus[:, :, 1])
                ar_in = dp.tile([128, M_T], f32, tag=f"ari{lname}", name=f"ari{lname}")
                ar_out = dp.tile([128, M_T], f32, tag=f"aro{lname}", name=f"aro{lname}")
                nc.sync.dma_start(out=ar_in[:], in_=pml[:])
                nc.gpsimd.collective_compute(
                    "AllReduce",
                    mybir.AluOpType.add,
                    replica_groups=[list(range(N_CORES))],
                    ins=[ar_in[:].opt()],
                    outs=[ar_out[:].opt()],
                )
                asb = sp.tile([128, M_T], f32, tag=f"asb{lname}", name=f"asb{lname}")
                nc.sync.dma_start(out=asb[:], in_=ar_out[:])
                nmu = sp.tile([128, M_T], f32, tag=f"nmu{lname}", name=f"nmu{lname}")
                nc.vector.tensor_scalar_mul(nmu[:], asb[:], -(1.0 / B_FULL))
                if debug_outputs:
                    nc.sync.dma_start(
                        out=dbg["dbg_nmu2" if lname == "1" else "dbg_nmu3"][:],
                        in_=nmu[:],
                    )
                return nmu

            # ---- layer 2 (fp8 DoubleRow, n-major; first 26 chunks staged
            # to fp16 so the PE never waits for the stats round-trip) ----
            fp16 = mybir.dt.float16

            def lx_mms_pair(ldr, h_in, n, m0, tagp):
                """Two DoubleRow chunks (m0, m0+1) with interleaved groups."""
                nsl = slice(n * NCHUNK, (n + 1) * NCHUNK)
                pss = [
                    yp.tile([128, NCHUNK], f32, tag="y", name=f"ps{tagp}_{n}_{m0 + j}")
                    for j in range(2)
                ]
                for a in range(A_T):
                    for j in range(2):
                        msl = slice((m0 + j) * 128, (m0 + j + 1) * 128)
                        nc.tensor.matmul(
                            pss[j][:],
                            ldr[a][:, :, msl],
                            h_in[a][:, :, nsl],
                            start=(a == 0),
                            stop=(a == A_T - 1),
                            perf_mode=DR,
                        )
                return pss

            sgt = sp.tile([128, NCHUNK], fp8, tag="sgt", name="sgt")
            slt = sp.tile([128, NCHUNK], fp8, tag="slt", name="slt")

            def dve_sign(dst_ap, src_ap, bias_ap, accum_ap):
                """sign(y + b) = (y+b > 0) - (y+b < 0) on the VectorE."""
                nc.vector.tensor_scalar(
                    sgt[:], src_ap, bias_ap, 0.0,
                    op0=mybir.AluOpType.add, op1=mybir.AluOpType.is_gt,
                )
                nc.vector.tensor_scalar(
                    slt[:], src_ap, bias_ap, 0.0,
                    op0=mybir.AluOpType.add, op1=mybir.AluOpType.is_lt,
                )
                nc.vector.scalar_tensor_tensor(
                    dst_ap, sgt[:], 0.0, slt[:],
                    op0=mybir.AluOpType.add, op1=mybir.AluOpType.subtract,
                    accum_out=accum_ap,
                )

            def sign2(src_ap, n, m, engine="act"):
                nsl = slice(n * NCHUNK, (n + 1) * NCHUNK)
                c = m * N_T + n
                if engine == "dve":
                    dve_sign(
                        h2p[m // 2][:, m % 2, nsl],
                        src_ap,
                        nmu2[:, m : m + 1],
                        hacc2[:, c : c + 1],
                    )
                else:
                    nc.scalar.activation(
                        h2p[m // 2][:, m % 2, nsl],
                        src_ap,
                        AF.Sign,
                        bias=nmu2[:, m : m + 1],
                        accum_out=hacc2[:, c : c + 1],
                    )

            pairs = [(n, m0) for n in range(N_T) for m0 in range(0, M_T, 2)]
            N_STG2P = 15  # staged chunk-pairs (30 chunks)
            staged2 = []

            def stage2(n, m0):
                pss = lx_mms_pair(s2dr, h1p, n, m0, "2")
                for j in range(2):
                    st = stp.tile(
                        [128, NCHUNK], fp16, tag="stg", name=f"st2_{n}_{m0 + j}"
                    )
                    # alternate copy engines: either alone can't keep up with
                    # the PE's chunk rate and would back up the PSUM pool
                    if j == 0:
                        nc.scalar.copy(st[:], pss[j][:])
                    else:
                        nc.vector.tensor_copy(st[:], pss[j][:])
                    staged2.append((st, n, m0 + j))

            stage2(*pairs[0])
            nmu2 = stats(hacc1, s2b, "1")
            for n, m0 in pairs[1:N_STG2P]:
                stage2(n, m0)
            for st, n, m in staged2:
                sign2(st[:], n, m)
            for n, m0 in pairs[N_STG2P:]:
                pss = lx_mms_pair(s2dr, h1p, n, m0, "2")
                for j in range(2):
                    sign2(pss[j][:], n, m0 + j)

            if debug_outputs:
                for a in range(A_T):
                    for j in range(2):
                        nc.sync.dma_start(
                            out=dbg["dbg_h2"][:, 2 * a + j, :], in_=h2p[a][:, j, :]
                        )

            # ---- layer 3 + layer 4 ------------------------------------
            h3c = {}  # n -> list of pair tiles

            def h3_tile(n, m):
                a = m // 2
                if n not in h3c:
                    h3c[n] = [None] * A_T
                if h3c[n][a] is None:
                    h3c[n][a] = h3p_pool.tile(
                        [128, 2, NCHUNK], fp8, tag=f"h3_{a}", name=f"h3_{a}_{n}"
                    )
                return h3c[n][a]

            def sign3(src_ap, n, m):
                t = h3_tile(n, m)
                nc.scalar.activation(
                    t[:, m % 2, :], src_ap, AF.Sign, bias=nmu3[:, m : m + 1]
                )

            def l4(n):
                nsl = slice(n * NCHUNK, (n + 1) * NCHUNK)
                p4 = lp.tile([16, NCHUNK], f32, tag="y4", name=f"p4_{n}")
                for a in range(A_T):
                    nc.tensor.matmul(
                        p4[:],
                        s4dr[a][:],
                        h3c[n][a][:],
                        start=(a == 0),
                        stop=(a == A_T - 1),
                        perf_mode=DR,
                    )
                oc = op_.tile([D_OUT, NCHUNK], f32, tag="oc", name=f"oc_{n}")
                nc.vector.tensor_copy(oc[:], p4[0:D_OUT, :])
                nc.sync.dma_start(out=out_d[:, nsl], in_=oc[:])

            N_STG3P = 15  # staged chunk-pairs (30 chunks), ring shared with L2
            staged3 = []

            def stage3(n, m0):
                pss = lx_mms_pair(s3dr, h2p, n, m0, "3")
                for j in range(2):
                    st = stp.tile(
                        [128, NCHUNK], fp16, tag="stg", name=f"st3_{n}_{m0 + j}"
                    )
                    if j == 0:
                        nc.scalar.copy(st[:], pss[j][:])
                    else:
                        nc.vector.tensor_copy(st[:], pss[j][:])
                    staged3.append((st, n, m0 + j))

            for n, m0 in pairs[:3]:
                stage3(n, m0)
            nmu3 = stats(hacc2, s3b, "2")
            for n, m0 in pairs[3:N_STG3P]:
                stage3(n, m0)
            for st, n, m in staged3:
                sign3(st[:], n, m)
            for n, m0 in pairs[N_STG3P:]:
                pss = lx_mms_pair(s3dr, h2p, n, m0, "3")
                for j in range(2):
                    sign3(pss[j][:], n, m0 + j)
            l4(0)
            l4(1)
            l4(2)
            l4(3)

    nc.compile()
    return nc


def _get_program():
    global _PROGRAM
    if _PROGRAM is None:
        _PROGRAM = _build_program()
    return _PROGRAM


def _split3_bf16(a32):
    """Split fp32 array into three bf16 terms summing exactly to a32."""
    a0 = a32.astype(BF16)
    r = a32 - a0.astype(np.float32)
    a1 = r.astype(BF16)
    r2 = r - a1.astype(np.float32)
    a2 = r2.astype(BF16)
    return a0, a1, a2


def _dr_layout(st, dout):
    """[D, dout] K-major sign matrix -> DoubleRow lhsT blocks
    [A_T, 128, 2, dout] with element (a, ki, j, q) = st[(2a+j)*128 + ki, q]."""
    return np.ascontiguousarray(
        st.reshape(A_T, 2, 128, dout).transpose(0, 2, 1, 3).astype(FP8)
    )


def _numpy_fallback(x, W1, g1, b1, W2, g2, b2, W3, g3, b3, W4):
    eps = np.float32(1e-5)

    def bn_sign(y, g, b):
        mu = y.mean(axis=0, dtype=np.float32)
        var = np.mean(np.square(y - mu), axis=0, dtype=np.float32)
        return np.sign(g * (y - mu) / np.sqrt(var + eps) + b).astype(np.float32)

    h = bn_sign(x @ np.sign(W1).T, g1, b1)
    h = bn_sign(h @ np.sign(W2).T, g2, b2)
    h = bn_sign(h @ np.sign(W3).T, g3, b3)
    return (h @ np.sign(W4).T).astype(np.float32)


def kernel(x, W1, g1, b1, W2, g2, b2, W3, g3, b3, W4):
    global LAST_RESULTS
    x = np.asarray(x, np.float32).reshape(-1, D_IN)
    args = [np.asarray(a, np.float32) for a in (W1, g1, b1, W2, g2, b2, W3, g3, b3, W4)]
    W1, g1, b1, W2, g2, b2, W3, g3, b3, W4 = args

    specializable = (
        x.shape == (B_FULL, D_IN)
        and all((g > 0).all() for g in (g1, g2, g3))
        and all((b == 0).all() for b in (b1, b2, b3))
    )
    if not specializable:
        return _numpy_fallback(x, W1, g1, b1, W2, g2, b2, W3, g3, b3, W4)

    from concourse.bass_utils import run_bass_kernel_spmd

    s1 = np.sign(W1)  # [1024, 784]
    s1t = np.zeros((K1, D), BF16)
    s1t[:D_IN, :] = s1.T.astype(BF16)
    s2t = np.ascontiguousarray(np.sign(W2).T)  # [in, out] f32
    s3t = np.ascontiguousarray(np.sign(W3).T)
    s4t = np.ascontiguousarray(np.sign(W4).T)

    xt = np.zeros((K1, B_FULL), np.float32)
    xt[:D_IN, :] = x.T
    x0, x1, x2 = _split3_bf16(xt)

    # layer-1 batch mean, computed on host in fp64:
    # mean(x @ S1.T, axis=0) == (S1 @ sum(x, axis=0)) / B
    xsum = x.sum(axis=0, dtype=np.float64)  # [784]
    mu1 = (s1.astype(np.float64) @ xsum) / float(B_FULL)  # [1024]
    negmu1 = np.ascontiguousarray(
        (-mu1).astype(np.float32).reshape(M_T, 128).T
    )  # [128, M_T], column m <-> features m*128 + p

    common = {
        "s1t": s1t,
        "s2b": np.ascontiguousarray(s2t.astype(BF16)),
        "s3b": np.ascontiguousarray(s3t.astype(BF16)),
        "s2dr": _dr_layout(s2t, D),
        "s3dr": _dr_layout(s3t, D),
        "s4dr": _dr_layout(np.concatenate([s4t, np.zeros((D, 6), s4t.dtype)], axis=1), 16),
        "negmu1": negmu1,
    }
    xs_all = np.stack([x0, x1, x2])  # [3, K1, B_FULL]
    in_maps = []
    for c in range(N_CORES):
        sl = slice(c * B_SHARD, (c + 1) * B_SHARD)
        in_maps.append(
            {
                "xs": np.ascontiguousarray(xs_all[:, :, sl]),
                **common,
            }
        )

    nc = _get_program()
    LAST_RESULTS = run_bass_kernel_spmd(nc, in_maps, core_ids=list(range(N_CORES)))
    y = np.concatenate(
        [LAST_RESULTS.results[c]["out"] for c in range(N_CORES)], axis=1
    )  # [10, 16384]
    return np.ascontiguousarray(y.T).astype(np.float32)

